# revision 7
# baseline (speedup 1.0000x reference)
"""Trainium2 Bass kernel for nn_LocalEnergyOpt (molecular-mechanics local energy), v2.

Per batch sample (B=128): features[:, :, 5] packs coords [4096, 3]; col 6 bonds
(i,j,t)x4095; col 7 angles (i,j,k,t)x4094; col 8 torsions (i,j,k,l,t)x4093.
  e_bond = opt[0] * sum k_t (|ci-cj| - r0_t)^2
  e_ang  = opt[1] * sum k_t (theta - th0_t)^2, theta = arccos(clip(cos))
  e_tor  = opt[2] * sum k_t (1 + cos(n_t phi - d_t)), phi = atan2(y, x)
Output [B, 3].

Sharding: pure data parallel, 16 samples per NeuronCore across 8 cores.

v2 pipeline per NC (2 waves x 8 samples; GPSIMD core g handles sample 8w+g on
partitions 16g..16g+15):
  stage flat features -> DVE stride-9/27 column+component extraction ->
  per-sample DRAM planes (x,y,z coord planes + packed index columns) ->
  coord gather table with COMPONENT-CYCLED partitions (partition 16g+3b+c
  holds component c of sample g) so ONE ap_gather per endpoint list yields
  x,y,z on adjacent partitions (idx list shared within group) -> dedup via
  SBUF->SBUF redistribution DMAs to dense [128, 256] planes -> plane-wise
  DVE/ACT energy pipeline -> masked reduce -> per-wave PE matmul -> scale.

Param tables use the same trick with period-2/period-4 partition cycling
(bond/angle: k,x0 planes; torsion: k,cos d,sin d,n planes) -> one param
ap_gather per class.

ap_gather unwrap is interleaved: out position T uses idx[T%16, T//16]; with
idx plane partition j = contiguous list chunk [256j, 256j+256) the dense
position (j, u) maps to list position 256*(u%16) + 16j + u//16, so pad-tail
masks kill (j==15, u%16==15, u//16 >= 16-ntail).
"""

import sys
import functools

import numpy as np

sys.path.insert(0, "/opt/trn_rl_repo")

from concourse import bacc, mybir  # noqa: E402
import concourse.tile as tile  # noqa: E402
from concourse.alu_op_type import AluOpType as Op  # noqa: E402

F32 = mybir.dt.float32
F16 = mybir.dt.float16
I16 = mybir.dt.int16
I32 = mybir.dt.int32
AF = mybir.ActivationFunctionType
AX = mybir.AxisListType

# Problem constants
N_CORES = 8
NS = 16                      # samples per NeuronCore
NB, NA, NT = 4095, 4094, 4093
NATOMS = 4096
MAXLEN = 20465
CR = 162                     # dense col rows per partition (mult of 3!)
LPP = 9 * CR                 # 1458 flat f32 per partition
FLATPAD = 128 * LPP          # 186624 >= 184185
COLN = 128 * CR              # 20736 dense col length
QPP = CR // 3                # 54 coord-plane entries per partition
PLN = 128 * QPP              # 6912 coord plane length
EPS = 1e-8
PI = float(np.pi)

LIST = 4096                  # per-sample index list length per class (padded)
DP = 256                     # dense positions per partition


def build_nc():
    nc = bacc.Bacc(None, target_bir_lowering=False, debug=False)

    feat = nc.dram_tensor("features", [NS, FLATPAD], F32, kind="ExternalInput")
    bond_t = nc.dram_tensor("bond_type", [15, 2], F32, kind="ExternalInput")
    ang_t = nc.dram_tensor("angle_type", [13, 2], F32, kind="ExternalInput")
    tor_t = nc.dram_tensor("tor_type", [25, 2], F32, kind="ExternalInput")
    mult_f = nc.dram_tensor("mult_f", [1, 25], F32, kind="ExternalInput")
    opt_p = nc.dram_tensor("opt_pars", [1, 47], F32, kind="ExternalInput")
    out_d = nc.dram_tensor("out", [NS, 3], F32, kind="ExternalOutput")

    with tile.TileContext(nc) as tc:
        with (
            tc.tile_pool(name="const", bufs=1) as constp,
            tc.tile_pool(name="stage", bufs=1) as stagep,
            tc.tile_pool(name="cext", bufs=2) as cextp,
            tc.tile_pool(name="tab", bufs=1) as tabp,
            tc.tile_pool(name="iraw", bufs=2) as irawp,
            tc.tile_pool(name="idx", bufs=2) as idxp,
            tc.tile_pool(name="gath", bufs=2) as gathp,
            tc.tile_pool(name="dn", bufs=2) as dnp,
            tc.tile_pool(name="dense", bufs=1) as densep,
            tc.tile_pool(name="work", bufs=1) as workp,
            tc.tile_pool(name="accs", bufs=1) as accp,
            tc.tile_pool(name="psum", bufs=1, space="PSUM") as psump,
            tc.tile_pool(name="dram", bufs=2, space="DRAM") as dramp,
            tc.tile_pool(name="dram1", bufs=1, space="DRAM") as dram1p,
        ):
            # ---------------- scalar constants ------------------------------
            fwork = constp.tile([128, 16], F32)
            cb = fwork[:, 0:4]
            nc.vector.memset(cb[:, 0:1], EPS)
            nc.vector.memset(cb[:, 1:2], PI / 2.0)
            nc.vector.memset(cb[:, 2:3], 1e-30)
            nc.vector.memset(cb[:, 3:4], -1.0)
            b_eps = cb[:, 0:1]
            b_pi2 = cb[:, 1:2]
            b_tiny = cb[:, 2:3]
            s_neg1 = cb[:, 3:4]

            # ---------------- masks + PE selector ---------------------------
            # layout: maskB [0:256] maskA [256:512] maskT [512:768] blk [768:776]
            cst = constp.tile([128, 776], F32)
            mB = cst[:, 0:DP]
            mA = cst[:, DP:2 * DP]
            mT = cst[:, 2 * DP:3 * DP]
            blk = cst[:, 3 * DP:3 * DP + 8]
            iwork = constp.tile([128, 780], I32)
            pidx = iwork[:, 0:1]
            colx = iwork[:, 1:257]
            i_a = iwork[:, 257:258]
            i_b = iwork[:, 258:259]
            u15 = iwork[:, 259:515]
            blki = iwork[:, 515:523]
            pdiv = iwork[:, 523:524]
            ush = iwork[:, 524:780]
            ftmp = constp.tile([128, 769], F32)
            row15 = ftmp[:, 0:1]
            tailf = ftmp[:, 1:257]
            uge = ftmp[:, 257:513]
            u15f = ftmp[:, 513:769]
            nc.gpsimd.iota(pidx, pattern=[[1, 1]], base=0, channel_multiplier=1)
            nc.gpsimd.iota(colx, pattern=[[1, 256]], base=0, channel_multiplier=0)
            nc.vector.tensor_scalar(out=i_a, in0=pidx, scalar1=15, scalar2=None,
                                    op0=Op.bitwise_and)
            nc.vector.tensor_scalar(out=i_b, in0=i_a, scalar1=15, scalar2=None,
                                    op0=Op.is_equal)
            nc.vector.tensor_copy(out=row15, in_=i_b)
            # u15 = (u & 15) == 15 ; uge_k = (u >> 4) >= 16-ntail
            nc.vector.tensor_scalar(out=u15, in0=colx, scalar1=15, scalar2=None,
                                    op0=Op.bitwise_and)
            nc.vector.tensor_scalar(out=u15, in0=u15, scalar1=15, scalar2=None,
                                    op0=Op.is_equal)
            nc.vector.tensor_copy(out=u15f, in_=u15)
            nc.vector.tensor_scalar(out=ush, in0=colx, scalar1=4, scalar2=None,
                                    op0=Op.arith_shift_right)
            for msk, ntail in ((mB, 1), (mA, 2), (mT, 3)):
                nc.vector.tensor_scalar(out=uge, in0=ush, scalar1=16 - ntail,
                                        scalar2=None, op0=Op.is_ge)
                nc.vector.tensor_tensor(out=tailf, in0=uge, in1=u15f, op=Op.mult)
                nc.vector.tensor_tensor(out=msk, in0=tailf,
                                        in1=row15.to_broadcast([128, DP]), op=Op.mult)
                nc.vector.tensor_scalar(out=msk, in0=msk, scalar1=-1.0, scalar2=1.0,
                                        op0=Op.mult, op1=Op.add)
            # selector: blk[p, c] = 1 iff p//16 == c
            nc.vector.tensor_scalar(out=pdiv, in0=pidx, scalar1=4, scalar2=None,
                                    op0=Op.arith_shift_right)
            nc.gpsimd.iota(blki, pattern=[[1, 8]], base=0, channel_multiplier=0)
            nc.vector.tensor_tensor(out=blki, in0=pdiv.to_broadcast([128, 8]),
                                    in1=blki, op=Op.is_equal)
            nc.vector.tensor_copy(out=blk, in_=blki)

            # ---------------- param tables (interleaved pairs) ---------------
            # bond/angle: (k, x0) pairs; torsion: (k, cos d) + (sin d, n).
            ptabs = constp.tile([128, 56], F32)
            ptabh = constp.tile([128, 100], F16)
            ptabh2 = constp.tile([128, 56], F16)
            ptth = ptabh[:]
            ptt = [ptth]
            pbt = [ptabh2[:, 0:30]]
            pat = [ptabh2[:, 30:56]]
            nc.sync.dma_start(
                out=ptabs[:, 0:30],
                in_=bond_t.ap().rearrange("a b -> (a b)")[None, :]
                .to_broadcast([128, 30]))
            nc.sync.dma_start(
                out=ptabs[:, 30:56],
                in_=ang_t.ap().rearrange("a b -> (a b)")[None, :]
                .to_broadcast([128, 26]))
            nc.vector.tensor_copy(out=ptabh2[:, 0:30], in_=ptabs[:, 0:30])
            nc.vector.tensor_copy(out=ptabh2[:, 30:56], in_=ptabs[:, 30:56])
            # torsion derived planes (k, cos d, sin d, n) built on partition 0
            onep = fwork[0:1, 4:16]
            t4w = constp.tile([1, 175], F32)
            traw = t4w[:, 0:50]
            mraw = t4w[:, 50:75]
            t4 = t4w[:, 75:175]
            nc.sync.dma_start(out=traw, in_=tor_t.ap().rearrange("a b -> (a b)")[None, :])
            nc.sync.dma_start(out=mraw, in_=mult_f.ap())
            t4q = t4.rearrange("p (n d) -> p n d", d=4)   # (k, cosd, sind, n)
            trv = traw.rearrange("p (n d) -> p n d", d=2)
            nc.vector.tensor_copy(out=t4q[:, :, 0], in_=trv[:, :, 0])           # k
            nc.scalar.activation(t4q[:, :, 1], trv[:, :, 1], AF.Sin,
                                 bias=b_pi2[0:1, :], scale=s_neg1[0:1, :])      # cos d
            nc.scalar.activation(t4q[:, :, 2], trv[:, :, 1], AF.Sin)            # sin d
            nc.vector.tensor_copy(out=t4q[:, :, 3], in_=mraw)                   # n
            t4h = onep  # reuse scratch region name
            t4h16 = constp.tile([1, 100], F16)
            nc.vector.tensor_copy(out=t4h16[:], in_=t4)
            t4_dram = dram1p.tile([1, 100], F16)
            nc.sync.dma_start(out=t4_dram[:], in_=t4h16[:])
            nc.sync.dma_start(
                out=ptth[:],
                in_=t4_dram[:].to_broadcast([128, 100]))

            acc6 = accp.tile([128, 6], F32)
            otmp = accp.tile([8, 6], F32)
            opt6 = constp.tile([8, 6], F32)
            nc.sync.dma_start(
                out=opt6[:],
                in_=opt_p.ap()[:, 0:3][:, None, :].to_broadcast([8, 2, 3]),
            )

            for w in range(2):
                # ------------- stage + column/component extraction ----------
                ccoord = dramp.tile([8, 4 * QPP * 128], F16, tag="ccoord")
                bcol = dramp.tile([8, COLN], F32, tag="bcol")
                acol = dramp.tile([8, COLN], F32, tag="acol")
                tcol = dramp.tile([8, COLN], F32, tag="tcol")
                for s8 in range(8):
                    s = 8 * w + s8
                    stage = stagep.tile([128, LPP], F32, tag="stage")
                    nc.sync.dma_start(
                        out=stage[:], in_=feat.ap()[s].rearrange("(p f) -> p f", f=LPP)
                    )
                    stv9 = stage[:].rearrange("p (r n) -> p r n", n=9)
                    cd = cextp.tile([128, 4 * CR], F32, tag="cd")
                    for k, (col, cdst) in enumerate(
                            ((6, bcol), (7, acol), (8, tcol))):
                        sl = cd[:, CR * (k + 1):CR * (k + 2)]
                        nc.vector.tensor_copy(out=sl, in_=stv9[:, :, col])
                        nc.scalar.dma_start(out=cdst[:][s8], in_=sl)
                    # coords -> fp16 (x,y,z,0) quads
                    nc.vector.tensor_copy(out=cd[:, 0:CR], in_=stv9[:, :, 5])
                    cdh = cextp.tile([128, 4 * QPP], F16, tag="cdh")
                    nc.vector.memset(cdh[:], 0.0)
                    nc.vector.tensor_copy(
                        out=cdh[:].rearrange("p (a k) -> p a k", k=4)[:, :, 0:3],
                        in_=cd[:, 0:CR].rearrange("p (a k) -> p a k", k=3))
                    nc.scalar.dma_start(out=ccoord[:][s8], in_=cdh[:])

                # ------------- fp16-quad coord table ------------------------
                # partition 16g+j holds sample g's (x,y,z,0) fp16 quads
                tab = tabp.tile([128, 4 * 4096], F16, tag="tab")
                nc.sync.dma_start(
                    out=tab[:],
                    in_=ccoord[:][:, None, 0:4 * NATOMS].to_broadcast(
                        [8, 16, 4 * NATOMS]),
                )

                # ------------- per-class: indices -> gathers -> energy ------
                for (ccol, ne, ptab, pn, pp, acc_c) in (
                    (bcol, 2, pbt, 15, 2, 0),
                    (acol, 3, pat, 13, 2, 1),
                    (tcol, 4, ptt, 25, 4, 2),
                ):
                    cnt = (ne + 1) * DP
                    iraw = irawp.tile([128, cnt], F32, tag="iraw")
                    nc.scalar.dma_start(
                        out=iraw[:],
                        in_=ccol[:][:, 0:16 * cnt].rearrange("s (j f) -> s j f", j=16),
                    )
                    idxt = idxp.tile([128, (ne + 1) * DP], I16, tag="idxt")
                    irv = iraw[:].rearrange("p (e k) -> p e k", k=ne + 1)
                    for e in range(ne + 1):
                        nc.vector.tensor_copy(
                            out=idxt[:, DP * e:DP * (e + 1)], in_=irv[:, :, e])

                    npl = 3 * ne + pp
                    pl = densep.tile([128, npl * DP], F32, tag=f"pl{acc_c}")

                    def plv(n):
                        return pl[:, DP * n:DP * (n + 1)]

                    # coord gathers: one d=4 fp16-quad gather per endpoint
                    for e in range(ne):
                        gh = gathp.tile([128, 4 * LIST], F16, tag="gh")
                        nc.gpsimd.ap_gather(
                            out_ap=gh[:].rearrange("p (n d) -> p n d", d=4),
                            in_ap=tab[:].rearrange("p (n d) -> p n d", d=4),
                            idxs_ap=idxt[:, DP * e:DP * (e + 1)],
                            channels=128,
                            num_elems=4096,
                            d=4,
                            num_idxs=LIST,
                        )
                        dnh = dnp.tile([128, 4 * DP], F16, tag="dnh")
                        nc.sync.dma_start(
                            out=dnh[:],
                            in_=gh[:].rearrange("(a b) f -> a b f", b=16)[:, 0, :],
                        )
                        dv = dnh[:].rearrange("p (n d) -> p n d", d=4)
                        for r in range(3):
                            nc.vector.tensor_copy(out=plv(3 * e + r), in_=dv[:, :, r])
                    if pp == 4:
                        # torsion params: one d=4 fp16 (k, cosd, sind, n) gather
                        gh = gathp.tile([128, 4 * LIST], F16, tag="gh")
                        nc.gpsimd.ap_gather(
                            out_ap=gh[:].rearrange("p (n d) -> p n d", d=4),
                            in_ap=ptab[0].rearrange("p (n d) -> p n d", d=4),
                            idxs_ap=idxt[:, DP * ne:DP * (ne + 1)],
                            channels=128,
                            num_elems=pn,
                            d=4,
                            num_idxs=LIST,
                        )
                        dnh = dnp.tile([128, 4 * DP], F16, tag="dnh")
                        nc.sync.dma_start(
                            out=dnh[:],
                            in_=gh[:].rearrange("(a b) f -> a b f", b=16)[:, 0, :],
                        )
                        dv = dnh[:].rearrange("p (n d) -> p n d", d=4)
                        for r in range(4):
                            nc.vector.tensor_copy(
                                out=plv(3 * ne + r), in_=dv[:, :, r])
                    else:
                        # bond/angle params: d=2 fp16 (k, x0) gather
                        gh = gathp.tile([128, 4 * LIST], F16, tag="gh")
                        g2 = gh[:, 0:2 * LIST]
                        nc.gpsimd.ap_gather(
                            out_ap=g2.rearrange("p (n d) -> p n d", d=2),
                            in_ap=ptab[0].rearrange("p (n d) -> p n d", d=2),
                            idxs_ap=idxt[:, DP * ne:DP * (ne + 1)],
                            channels=128,
                            num_elems=pn,
                            d=2,
                            num_idxs=LIST,
                        )
                        dnh = dnp.tile([128, 4 * DP], F16, tag="dnh")
                        dn2 = dnh[:, 0:2 * DP]
                        nc.sync.dma_start(
                            out=dn2,
                            in_=g2.rearrange("(a b) f -> a b f", b=16)[:, 0, :],
                        )
                        dv = dn2.rearrange("p (n d) -> p n d", d=2)
                        for r in range(2):
                            nc.vector.tensor_copy(
                                out=plv(3 * ne + r), in_=dv[:, :, r])

                    acc = acc6[:, 3 * w + acc_c:3 * w + acc_c + 1]
                    if acc_c == 0:
                        bond_energy(nc, workp, plv, pl, mB, b_eps, acc)
                    elif acc_c == 1:
                        angle_energy(nc, workp, plv, pl, mA, b_eps, acc)
                    else:
                        torsion_energy(nc, workp, plv, pl, mT, b_eps, b_tiny, acc)

            # ------------- final reduction: [128, 6] -> [8, 6] -> out -------
            pacc = psump.tile([8, 6], F32, tag="pacc")
            nc.tensor.matmul(out=pacc[:], lhsT=blk, rhs=acc6[:], start=True, stop=True)
            nc.vector.tensor_copy(out=otmp[:], in_=pacc[:])
            nc.vector.tensor_mul(out=otmp[:], in0=otmp[:], in1=opt6[:])
            nc.sync.dma_start(out=out_d.ap()[0:8, :], in_=otmp[:, 0:3])
            nc.sync.dma_start(out=out_d.ap()[8:16, :], in_=otmp[:, 3:6])

    nc.compile()
    return nc


def bond_energy(nc, workp, plv, pl, mB, b_eps, acc):
    # planes: xi yi zi (0:3) xj yj zj (3:6) k (6) r0 (7)
    wb = workp.tile([128, 8 * DP], F32, tag="w8")
    d3 = wb[:, 0:3 * DP]
    sq = wb[:, 3 * DP:6 * DP]
    r2 = wb[:, 6 * DP:7 * DP]
    scr = wb[:, 7 * DP:8 * DP]
    nc.vector.tensor_sub(out=d3, in0=pl[:, 0:3 * DP], in1=pl[:, 3 * DP:6 * DP])
    nc.vector.tensor_mul(out=sq, in0=d3, in1=d3)
    nc.vector.tensor_reduce(
        out=r2, in_=sq.rearrange("p (c u) -> p u c", c=3), axis=AX.X, op=Op.add)
    r = wb[:, 0:DP]                       # d3 dead
    nc.scalar.activation(r, r2, AF.Sqrt, bias=b_eps)
    u = wb[:, DP:2 * DP]
    nc.vector.tensor_sub(out=u, in0=r, in1=plv(7))
    e = wb[:, 2 * DP:3 * DP]
    nc.scalar.activation(e, u, AF.Square)
    km = wb[:, 3 * DP:4 * DP]
    nc.vector.tensor_tensor(out=km, in0=plv(6), in1=mB, op=Op.mult)
    nc.vector.tensor_mul(out=scr, in0=e, in1=km)
    nc.vector.tensor_reduce(out=acc, in_=scr, axis=AX.X, op=Op.add)


def angle_energy(nc, workp, plv, pl, mA, b_eps, acc):
    # planes: i (0:3) j (3:6) k (6:9) k_t (9) th0 (10)
    wv = workp.tile([128, 6 * DP], F32, tag="w6")
    v1 = wv[:, 0:3 * DP]
    v2 = wv[:, 3 * DP:6 * DP]
    nc.vector.tensor_sub(out=v1, in0=pl[:, 0:3 * DP], in1=pl[:, 3 * DP:6 * DP])
    nc.vector.tensor_sub(out=v2, in0=pl[:, 6 * DP:9 * DP], in1=pl[:, 3 * DP:6 * DP])
    wa = workp.tile([128, 8 * DP], F32, tag="w8")
    prod = wa[:, 0:3 * DP]
    d11 = wa[:, 3 * DP:4 * DP]
    d22 = wa[:, 4 * DP:5 * DP]
    d12 = wa[:, 5 * DP:6 * DP]

    def dot3(dst, a, b):
        nc.vector.tensor_mul(out=prod, in0=a, in1=b)
        nc.vector.tensor_reduce(
            out=dst, in_=prod.rearrange("p (c u) -> p u c", c=3), axis=AX.X, op=Op.add)

    dot3(d11, v1, v1)
    dot3(d22, v2, v2)
    dot3(d12, v1, v2)
    s1 = wa[:, 0:DP]                      # prod dead
    s2a = wa[:, 1 * DP:2 * DP]
    nc.scalar.activation(s1, d11, AF.Sqrt, bias=b_eps)
    nc.scalar.activation(s2a, d22, AF.Sqrt, bias=b_eps)
    den = wa[:, 2 * DP:3 * DP]
    nc.vector.tensor_mul(out=den, in0=s1, in1=s2a)
    nc.vector.reciprocal(out=den, in_=den)
    cosv = wa[:, 3 * DP:4 * DP]           # d11 dead
    nc.vector.tensor_mul(out=cosv, in0=d12, in1=den)
    cosc = wa[:, 4 * DP:5 * DP]           # d22 dead
    nc.vector.tensor_scalar(
        out=cosc, in0=cosv, scalar1=-1.0 + 1e-6, scalar2=1.0 - 1e-6,
        op0=Op.max, op1=Op.min)
    # theta = arccos(cosc) via two bounded-arg arctan branches:
    #  |c| >  s: theta = arctan(s/c) + pi*(c<0)
    #  |c| <= s: theta = pi/2 - arctan(c/s), s = sqrt(1-c^2)
    cc = wa[:, 0:DP]                      # s1 dead
    nc.scalar.activation(cc, cosc, AF.Square)
    om = wa[:, 1 * DP:2 * DP]             # s2a dead
    nc.vector.tensor_scalar(
        out=om, in0=cc, scalar1=-1.0, scalar2=1.0, op0=Op.mult, op1=Op.add)
    sn = wa[:, 2 * DP:3 * DP]             # den dead
    nc.scalar.activation(sn, om, AF.Sqrt)
    sgn = wa[:, 1 * DP:2 * DP]            # om dead
    nc.vector.tensor_scalar(
        out=sgn, in0=cosc, scalar1=0.0, scalar2=None, op0=Op.is_ge)
    nc.vector.tensor_scalar(
        out=sgn, in0=sgn, scalar1=2e-18, scalar2=-1e-18, op0=Op.mult, op1=Op.add)
    csafe = wa[:, 5 * DP:6 * DP]          # d12 dead
    nc.vector.tensor_add(out=csafe, in0=cosc, in1=sgn)
    nc.vector.reciprocal(out=csafe, in_=csafe)
    ra = wa[:, 1 * DP:2 * DP]             # sgn dead
    nc.vector.tensor_mul(out=ra, in0=sn, in1=csafe)
    nc.vector.tensor_scalar(
        out=ra, in0=ra, scalar1=-1.0, scalar2=1.0, op0=Op.max, op1=Op.min)
    ata = wa[:, 5 * DP:6 * DP]            # csafe dead
    nc.scalar.activation(ata, ra, AF.Arctan)
    corr = wa[:, 1 * DP:2 * DP]           # ra dead
    nc.vector.tensor_scalar(
        out=corr, in0=cosc, scalar1=0.0, scalar2=PI, op0=Op.is_lt, op1=Op.mult)
    tha = wa[:, 6 * DP:7 * DP]
    nc.vector.tensor_add(out=tha, in0=ata, in1=corr)
    nc.vector.reciprocal(out=sn, in_=sn)
    rb = wa[:, 1 * DP:2 * DP]             # corr dead
    nc.vector.tensor_mul(out=rb, in0=cosc, in1=sn)
    nc.vector.tensor_scalar(
        out=rb, in0=rb, scalar1=-1.0, scalar2=1.0, op0=Op.max, op1=Op.min)
    thb = wa[:, 5 * DP:6 * DP]            # ata dead
    nc.scalar.activation(thb, rb, AF.Arctan)
    nc.vector.tensor_scalar(
        out=thb, in0=thb, scalar1=-1.0, scalar2=PI / 2.0, op0=Op.mult, op1=Op.add)
    wi = workp.tile([128, DP], I32, tag="wi")
    nc.vector.tensor_scalar(
        out=wi[:], in0=cc, scalar1=0.5, scalar2=None, op0=Op.is_gt)
    th = wa[:, 7 * DP:8 * DP]
    nc.vector.select(out=th, mask=wi[:], on_true=tha, on_false=thb)
    ua = wa[:, 0:DP]                      # cc dead
    nc.vector.tensor_sub(out=ua, in0=th, in1=plv(10))
    ea = wa[:, 1 * DP:2 * DP]             # rb dead
    nc.scalar.activation(ea, ua, AF.Square)
    kma = wa[:, 2 * DP:3 * DP]            # sn dead
    nc.vector.tensor_tensor(out=kma, in0=plv(9), in1=mA, op=Op.mult)
    scr = wa[:, 3 * DP:4 * DP]            # cosv dead
    nc.vector.tensor_mul(out=scr, in0=ea, in1=kma)
    nc.vector.tensor_reduce(out=acc, in_=scr, axis=AX.X, op=Op.add)


def torsion_energy(nc, workp, plv, pl, mT, b_eps, b_tiny, acc):
    # planes: i (0:3) j (3:6) k (6:9) l (9:12) k_t (12) cosd (13) sind (14) n (15)
    wv = workp.tile([128, 6 * DP], F32, tag="w6")
    wb9 = workp.tile([128, 9 * DP], F32, tag="w9")
    b1 = wb9[:, 0:3 * DP]
    b2 = wb9[:, 3 * DP:6 * DP]
    b3 = wb9[:, 6 * DP:9 * DP]
    nc.vector.tensor_sub(out=b1, in0=pl[:, 3 * DP:6 * DP], in1=pl[:, 0:3 * DP])
    nc.vector.tensor_sub(out=b2, in0=pl[:, 6 * DP:9 * DP], in1=pl[:, 3 * DP:6 * DP])
    nc.vector.tensor_sub(out=b3, in0=pl[:, 9 * DP:12 * DP], in1=pl[:, 6 * DP:9 * DP])

    def comp(t, m):
        return t[:, DP * m:DP * (m + 1)]

    # n1 = b1 x b2 -> wv[0:3]; n2 = b2 x b3 -> wv[3:6]
    wt = workp.tile([128, 8 * DP], F32, tag="w8")
    t0 = wt[:, 0:DP]
    t1 = wt[:, DP:2 * DP]
    for m in range(3):
        m1_, m2_ = (m + 1) % 3, (m + 2) % 3
        nc.vector.tensor_mul(out=t0, in0=comp(b1, m1_), in1=comp(b2, m2_))
        nc.vector.tensor_mul(out=t1, in0=comp(b1, m2_), in1=comp(b2, m1_))
        nc.vector.tensor_sub(out=comp(wv, m), in0=t0, in1=t1)
        nc.vector.tensor_mul(out=t0, in0=comp(b2, m1_), in1=comp(b3, m2_))
        nc.vector.tensor_mul(out=t1, in0=comp(b2, m2_), in1=comp(b3, m1_))
        nc.vector.tensor_sub(out=comp(wv, 3 + m), in0=t0, in1=t1)
    n1 = wv[:, 0:3 * DP]
    n2 = wv[:, 3 * DP:6 * DP]
    # q2 = |b2|^2 (b1 slot becomes scratch after m1 computed; use b3 later)
    q2 = wt[:, 2 * DP:3 * DP]
    sq = workp.tile([128, 3 * DP], F32, tag="w3")
    nc.vector.tensor_mul(out=sq[:], in0=b2, in1=b2)
    nc.vector.tensor_reduce(
        out=q2, in_=sq[:].rearrange("p (c u) -> p u c", c=3), axis=AX.X, op=Op.add)
    # m1' = n1 x b2 -> b3 slot (b3 dead after n2)
    for m in range(3):
        m1_, m2_ = (m + 1) % 3, (m + 2) % 3
        nc.vector.tensor_mul(out=t0, in0=comp(n1, m1_), in1=comp(b2, m2_))
        nc.vector.tensor_mul(out=t1, in0=comp(n1, m2_), in1=comp(b2, m1_))
        nc.vector.tensor_sub(out=comp(b3, m), in0=t0, in1=t1)
    mp = b3
    # X = n1.n2 ; Yr = m1'.n2
    X = wt[:, 3 * DP:4 * DP]
    Yr = wt[:, 4 * DP:5 * DP]
    nc.vector.tensor_mul(out=sq[:], in0=n1, in1=n2)
    nc.vector.tensor_reduce(
        out=X, in_=sq[:].rearrange("p (c u) -> p u c", c=3), axis=AX.X, op=Op.add)
    nc.vector.tensor_mul(out=sq[:], in0=mp, in1=n2)
    nc.vector.tensor_reduce(
        out=Yr, in_=sq[:].rearrange("p (c u) -> p u c", c=3), axis=AX.X, op=Op.add)
    rn = wt[:, 5 * DP:6 * DP]
    nc.scalar.activation(rn, q2, AF.Sqrt, bias=b_eps)
    nc.vector.reciprocal(out=rn, in_=rn)
    y = wt[:, 6 * DP:7 * DP]
    nc.vector.tensor_mul(out=y, in0=Yr, in1=rn)
    hx = wt[:, 2 * DP:3 * DP]             # q2 dead
    hy = wt[:, 4 * DP:5 * DP]             # Yr dead
    nc.scalar.activation(hx, X, AF.Square)
    nc.scalar.activation(hy, y, AF.Square)
    h = wt[:, 5 * DP:6 * DP]              # rn dead
    nc.vector.tensor_add(out=h, in0=hx, in1=hy)
    rh = wt[:, 2 * DP:3 * DP]             # hx dead
    nc.scalar.activation(rh, h, AF.Sqrt, bias=b_tiny)
    nc.vector.reciprocal(out=rh, in_=rh)
    c = wt[:, 4 * DP:5 * DP]              # hy dead
    s = wt[:, 5 * DP:6 * DP]              # h dead
    nc.vector.tensor_mul(out=c, in0=X, in1=rh)
    nc.vector.tensor_mul(out=s, in0=y, in1=rh)
    # Chebyshev: cos/sin of 2phi, 3phi (scratch: sq planes + X/rh/t0/t1 slots)
    cc = comp(sq, 0)
    c2 = comp(sq, 1)
    s2 = comp(sq, 2)
    sc = wt[:, 3 * DP:4 * DP]             # X dead
    c3 = wt[:, 2 * DP:3 * DP]             # rh dead
    s3 = wt[:, 6 * DP:7 * DP]             # y dead
    nc.scalar.activation(cc, c, AF.Square)
    nc.vector.tensor_scalar(
        out=c2, in0=cc, scalar1=2.0, scalar2=-1.0, op0=Op.mult, op1=Op.add)
    nc.vector.tensor_mul(out=sc, in0=s, in1=c)
    nc.vector.tensor_scalar(
        out=s2, in0=sc, scalar1=2.0, scalar2=None, op0=Op.mult)
    nc.vector.tensor_scalar(
        out=t0, in0=cc, scalar1=4.0, scalar2=-3.0, op0=Op.mult, op1=Op.add)
    nc.vector.tensor_mul(out=c3, in0=t0, in1=c)
    nc.vector.tensor_scalar(
        out=t0, in0=cc, scalar1=4.0, scalar2=-1.0, op0=Op.mult, op1=Op.add)
    nc.vector.tensor_mul(out=s3, in0=t0, in1=s)
    wi = workp.tile([128, 2 * DP], I32, tag="wi2")
    m2m = wi[:, 0:DP]
    m3m = wi[:, DP:2 * DP]
    nc.vector.tensor_scalar(
        out=m2m, in0=plv(15), scalar1=2.0, scalar2=None, op0=Op.is_equal)
    nc.vector.tensor_scalar(
        out=m3m, in0=plv(15), scalar1=3.0, scalar2=None, op0=Op.is_equal)
    cn = wt[:, 0:DP]                      # t0 dead
    sn = wt[:, 1 * DP:2 * DP]             # t1 dead
    nc.vector.select(out=cn, mask=m2m, on_true=c2, on_false=c)
    nc.vector.select(out=cn, mask=m3m, on_true=c3, on_false=cn)
    nc.vector.select(out=sn, mask=m2m, on_true=s2, on_false=s)
    nc.vector.select(out=sn, mask=m3m, on_true=s3, on_false=sn)
    tt1 = wt[:, 3 * DP:4 * DP]            # sc dead
    tt2 = wt[:, 4 * DP:5 * DP]            # c dead
    nc.vector.tensor_mul(out=tt1, in0=cn, in1=plv(13))
    nc.vector.tensor_mul(out=tt2, in0=sn, in1=plv(14))
    esum = wt[:, 5 * DP:6 * DP]           # s dead
    nc.vector.tensor_add(out=esum, in0=tt1, in1=tt2)
    nc.vector.tensor_scalar(
        out=esum, in0=esum, scalar1=1.0, scalar2=None, op0=Op.add)
    kmt = wt[:, 6 * DP:7 * DP]            # c3... s3 dead
    nc.vector.tensor_tensor(out=kmt, in0=plv(12), in1=mT, op=Op.mult)
    scr = wt[:, 7 * DP:8 * DP]
    nc.vector.tensor_mul(out=scr, in0=esum, in1=kmt)
    nc.vector.tensor_reduce(out=acc, in_=scr, axis=AX.X, op=Op.add)


@functools.lru_cache(maxsize=1)
def _get_nc():
    return build_nc()


def make_in_maps(inputs):
    """Shard full inputs into 8 per-core input maps."""
    feats = np.ascontiguousarray(inputs["features"], dtype=np.float32)
    Bf = feats.shape[0]
    flat = feats.reshape(Bf, -1)
    flat = np.concatenate(
        [flat, np.zeros((Bf, FLATPAD - flat.shape[1]), np.float32)], axis=1
    )
    bond_type = np.ascontiguousarray(inputs["bond_type"], np.float32)
    angle_type = np.ascontiguousarray(inputs["angle_type"], np.float32)
    tor_type = np.ascontiguousarray(inputs["tor_type"], np.float32)
    mult_f = np.ascontiguousarray(inputs["multiplicity"], np.float32).reshape(1, 25)
    opt = np.ascontiguousarray(inputs["opt_pars"], np.float32).reshape(1, 47)
    n_nc = Bf // NS
    in_maps = []
    for k in range(n_nc):
        in_maps.append({
            "features": flat[NS * k:NS * (k + 1)],
            "bond_type": bond_type,
            "angle_type": angle_type,
            "tor_type": tor_type,
            "mult_f": mult_f,
            "opt_pars": opt,
        })
    return in_maps


def kernel(**inputs) -> np.ndarray:
    from concourse.bass_utils import run_bass_kernel_spmd

    nc = _get_nc()
    in_maps = make_in_maps(inputs)
    res = run_bass_kernel_spmd(nc, in_maps, core_ids=list(range(len(in_maps))))
    outs = [res.results[k]["out"] for k in range(len(in_maps))]
    return np.concatenate(outs, axis=0).astype(np.float32)


def simulate_one_core(inputs, nc=None, trace=False):
    """CoreSim a single NC on the first 16 samples (for correctness dev)."""
    import concourse.bass_interp as bass_interp

    if nc is None:
        nc = _get_nc()
    in_map = make_in_maps(inputs)[0]
    sim = bass_interp.MultiCoreSim(nc, 1, trace=trace)
    for name, val in in_map.items():
        sim.cores[0].tensor(name)[:] = val
    sim.simulate(check_with_hw=False)
    return np.array(sim.cores[0].mem_tensor("out"))


if __name__ == "__main__":
    nc = build_nc()
    print("build ok")


# revision 8
# speedup vs baseline: 1.0690x; 1.0690x over previous
"""Trainium2 Bass kernel for nn_LocalEnergyOpt (molecular-mechanics local energy), v2.

Per batch sample (B=128): features[:, :, 5] packs coords [4096, 3]; col 6 bonds
(i,j,t)x4095; col 7 angles (i,j,k,t)x4094; col 8 torsions (i,j,k,l,t)x4093.
  e_bond = opt[0] * sum k_t (|ci-cj| - r0_t)^2
  e_ang  = opt[1] * sum k_t (theta - th0_t)^2, theta = arccos(clip(cos))
  e_tor  = opt[2] * sum k_t (1 + cos(n_t phi - d_t)), phi = atan2(y, x)
Output [B, 3].

Sharding: pure data parallel, 16 samples per NeuronCore across 8 cores.

v2 pipeline per NC (2 waves x 8 samples; GPSIMD core g handles sample 8w+g on
partitions 16g..16g+15):
  stage flat features -> DVE stride-9/27 column+component extraction ->
  per-sample DRAM planes (x,y,z coord planes + packed index columns) ->
  coord gather table with COMPONENT-CYCLED partitions (partition 16g+3b+c
  holds component c of sample g) so ONE ap_gather per endpoint list yields
  x,y,z on adjacent partitions (idx list shared within group) -> dedup via
  SBUF->SBUF redistribution DMAs to dense [128, 256] planes -> plane-wise
  DVE/ACT energy pipeline -> masked reduce -> per-wave PE matmul -> scale.

Param tables use the same trick with period-2/period-4 partition cycling
(bond/angle: k,x0 planes; torsion: k,cos d,sin d,n planes) -> one param
ap_gather per class.

ap_gather unwrap is interleaved: out position T uses idx[T%16, T//16]; with
idx plane partition j = contiguous list chunk [256j, 256j+256) the dense
position (j, u) maps to list position 256*(u%16) + 16j + u//16, so pad-tail
masks kill (j==15, u%16==15, u//16 >= 16-ntail).
"""

import sys
import functools

import numpy as np

sys.path.insert(0, "/opt/trn_rl_repo")

from concourse import bacc, mybir  # noqa: E402
import concourse.tile as tile  # noqa: E402
from concourse.alu_op_type import AluOpType as Op  # noqa: E402

F32 = mybir.dt.float32
F16 = mybir.dt.float16
I16 = mybir.dt.int16
I32 = mybir.dt.int32
AF = mybir.ActivationFunctionType
AX = mybir.AxisListType

# Problem constants
N_CORES = 8
NS = 16                      # samples per NeuronCore
NB, NA, NT = 4095, 4094, 4093
NATOMS = 4096
MAXLEN = 20465
CR = 162                     # dense col rows per partition (mult of 3!)
LPP = 9 * CR                 # 1458 flat f32 per partition
FLATPAD = 128 * LPP          # 186624 >= 184185
COLN = 128 * CR              # 20736 dense col length
QPP = CR // 3                # 54 coord-plane entries per partition
PLN = 128 * QPP              # 6912 coord plane length
EPS = 1e-8
PI = float(np.pi)

LIST = 4096                  # per-sample index list length per class (padded)
DP = 256                     # dense positions per partition


def build_nc():
    nc = bacc.Bacc(None, target_bir_lowering=False, debug=False)

    feat = nc.dram_tensor("features", [NS, FLATPAD], F32, kind="ExternalInput")
    bond_t = nc.dram_tensor("bond_type", [15, 2], F32, kind="ExternalInput")
    ang_t = nc.dram_tensor("angle_type", [13, 2], F32, kind="ExternalInput")
    tor_t = nc.dram_tensor("tor_type", [25, 2], F32, kind="ExternalInput")
    mult_f = nc.dram_tensor("mult_f", [1, 25], F32, kind="ExternalInput")
    opt_p = nc.dram_tensor("opt_pars", [1, 47], F32, kind="ExternalInput")
    out_d = nc.dram_tensor("out", [NS, 3], F32, kind="ExternalOutput")

    with tile.TileContext(nc) as tc:
        with (
            tc.tile_pool(name="const", bufs=1) as constp,
            tc.tile_pool(name="stage", bufs=1) as stagep,
            tc.tile_pool(name="cext", bufs=2) as cextp,
            tc.tile_pool(name="tab", bufs=1) as tabp,
            tc.tile_pool(name="iraw", bufs=2) as irawp,
            tc.tile_pool(name="idx", bufs=2) as idxp,
            tc.tile_pool(name="gath", bufs=2) as gathp,
            tc.tile_pool(name="dn", bufs=3) as dnp,
            tc.tile_pool(name="dense", bufs=1) as densep,
            tc.tile_pool(name="work", bufs=1) as workp,
            tc.tile_pool(name="accs", bufs=1) as accp,
            tc.tile_pool(name="psum", bufs=1, space="PSUM") as psump,
            tc.tile_pool(name="dram", bufs=2, space="DRAM") as dramp,
            tc.tile_pool(name="dram1", bufs=1, space="DRAM") as dram1p,
        ):
            # ---------------- scalar constants ------------------------------
            fwork = constp.tile([128, 16], F32)
            cb = fwork[:, 0:4]
            nc.vector.memset(cb[:, 0:1], EPS)
            nc.vector.memset(cb[:, 1:2], PI / 2.0)
            nc.vector.memset(cb[:, 2:3], 1e-30)
            nc.vector.memset(cb[:, 3:4], -1.0)
            b_eps = cb[:, 0:1]
            b_pi2 = cb[:, 1:2]
            b_tiny = cb[:, 2:3]
            s_neg1 = cb[:, 3:4]

            # ---------------- masks + PE selector ---------------------------
            # layout: maskB [0:256] maskA [256:512] maskT [512:768] blk [768:776]
            cst = constp.tile([128, 776], F32)
            mB = cst[:, 0:DP]
            mA = cst[:, DP:2 * DP]
            mT = cst[:, 2 * DP:3 * DP]
            blk = cst[:, 3 * DP:3 * DP + 8]
            iwork = constp.tile([128, 780], I32)
            pidx = iwork[:, 0:1]
            colx = iwork[:, 1:257]
            i_a = iwork[:, 257:258]
            i_b = iwork[:, 258:259]
            u15 = iwork[:, 259:515]
            blki = iwork[:, 515:523]
            pdiv = iwork[:, 523:524]
            ush = iwork[:, 524:780]
            ftmp = constp.tile([128, 769], F32)
            row15 = ftmp[:, 0:1]
            tailf = ftmp[:, 1:257]
            uge = ftmp[:, 257:513]
            u15f = ftmp[:, 513:769]
            nc.gpsimd.iota(pidx, pattern=[[1, 1]], base=0, channel_multiplier=1)
            nc.gpsimd.iota(colx, pattern=[[1, 256]], base=0, channel_multiplier=0)
            nc.vector.tensor_scalar(out=i_a, in0=pidx, scalar1=15, scalar2=None,
                                    op0=Op.bitwise_and)
            nc.vector.tensor_scalar(out=i_b, in0=i_a, scalar1=15, scalar2=None,
                                    op0=Op.is_equal)
            nc.vector.tensor_copy(out=row15, in_=i_b)
            # u15 = (u & 15) == 15 ; uge_k = (u >> 4) >= 16-ntail
            nc.vector.tensor_scalar(out=u15, in0=colx, scalar1=15, scalar2=None,
                                    op0=Op.bitwise_and)
            nc.vector.tensor_scalar(out=u15, in0=u15, scalar1=15, scalar2=None,
                                    op0=Op.is_equal)
            nc.vector.tensor_copy(out=u15f, in_=u15)
            nc.vector.tensor_scalar(out=ush, in0=colx, scalar1=4, scalar2=None,
                                    op0=Op.arith_shift_right)
            for msk, ntail in ((mB, 1), (mA, 2), (mT, 3)):
                nc.vector.tensor_scalar(out=uge, in0=ush, scalar1=16 - ntail,
                                        scalar2=None, op0=Op.is_ge)
                nc.vector.tensor_tensor(out=tailf, in0=uge, in1=u15f, op=Op.mult)
                nc.vector.tensor_tensor(out=msk, in0=tailf,
                                        in1=row15.to_broadcast([128, DP]), op=Op.mult)
                nc.vector.tensor_scalar(out=msk, in0=msk, scalar1=-1.0, scalar2=1.0,
                                        op0=Op.mult, op1=Op.add)
            # selector: blk[p, c] = 1 iff p//16 == c
            nc.vector.tensor_scalar(out=pdiv, in0=pidx, scalar1=4, scalar2=None,
                                    op0=Op.arith_shift_right)
            nc.gpsimd.iota(blki, pattern=[[1, 8]], base=0, channel_multiplier=0)
            nc.vector.tensor_tensor(out=blki, in0=pdiv.to_broadcast([128, 8]),
                                    in1=blki, op=Op.is_equal)
            nc.vector.tensor_copy(out=blk, in_=blki)

            # ---------------- param tables (interleaved pairs) ---------------
            # bond/angle: (k, x0) pairs; torsion: (k, cos d) + (sin d, n).
            ptabs = constp.tile([128, 56], F32)
            ptabh = constp.tile([128, 100], F16)
            ptabh2 = constp.tile([128, 56], F16)
            ptth = ptabh[:]
            ptt = [ptth]
            pbt = [ptabh2[:, 0:30]]
            pat = [ptabh2[:, 30:56]]
            nc.sync.dma_start(
                out=ptabs[:, 0:30],
                in_=bond_t.ap().rearrange("a b -> (a b)")[None, :]
                .to_broadcast([128, 30]))
            nc.sync.dma_start(
                out=ptabs[:, 30:56],
                in_=ang_t.ap().rearrange("a b -> (a b)")[None, :]
                .to_broadcast([128, 26]))
            nc.vector.tensor_copy(out=ptabh2[:, 0:30], in_=ptabs[:, 0:30])
            nc.vector.tensor_copy(out=ptabh2[:, 30:56], in_=ptabs[:, 30:56])
            # torsion derived planes (k, cos d, sin d, n) built on partition 0
            onep = fwork[0:1, 4:16]
            t4w = constp.tile([1, 175], F32)
            traw = t4w[:, 0:50]
            mraw = t4w[:, 50:75]
            t4 = t4w[:, 75:175]
            nc.sync.dma_start(out=traw, in_=tor_t.ap().rearrange("a b -> (a b)")[None, :])
            nc.sync.dma_start(out=mraw, in_=mult_f.ap())
            t4q = t4.rearrange("p (n d) -> p n d", d=4)   # (k, cosd, sind, n)
            trv = traw.rearrange("p (n d) -> p n d", d=2)
            nc.vector.tensor_copy(out=t4q[:, :, 0], in_=trv[:, :, 0])           # k
            nc.scalar.activation(t4q[:, :, 1], trv[:, :, 1], AF.Sin,
                                 bias=b_pi2[0:1, :], scale=s_neg1[0:1, :])      # cos d
            nc.scalar.activation(t4q[:, :, 2], trv[:, :, 1], AF.Sin)            # sin d
            nc.vector.tensor_copy(out=t4q[:, :, 3], in_=mraw)                   # n
            t4h = onep  # reuse scratch region name
            t4h16 = constp.tile([1, 100], F16)
            nc.vector.tensor_copy(out=t4h16[:], in_=t4)
            t4_dram = dram1p.tile([1, 100], F16)
            nc.sync.dma_start(out=t4_dram[:], in_=t4h16[:])
            nc.sync.dma_start(
                out=ptth[:],
                in_=t4_dram[:].to_broadcast([128, 100]))

            acc6 = accp.tile([128, 6], F32)
            otmp = accp.tile([8, 6], F32)
            opt6 = constp.tile([8, 6], F32)
            nc.sync.dma_start(
                out=opt6[:],
                in_=opt_p.ap()[:, 0:3][:, None, :].to_broadcast([8, 2, 3]),
            )

            for w in range(2):
                # ------------- stage + column/component extraction ----------
                ccoord = dramp.tile([8, 4 * QPP * 128], F16, tag="ccoord")
                bcol = dramp.tile([8, COLN], F32, tag="bcol")
                acol = dramp.tile([8, COLN], F32, tag="acol")
                tcol = dramp.tile([8, COLN], F32, tag="tcol")
                for s8 in range(8):
                    s = 8 * w + s8
                    stage = stagep.tile([128, LPP], F32, tag="stage")
                    nc.sync.dma_start(
                        out=stage[:], in_=feat.ap()[s].rearrange("(p f) -> p f", f=LPP)
                    )
                    stv9 = stage[:].rearrange("p (r n) -> p r n", n=9)
                    cd = cextp.tile([128, 4 * CR], F32, tag="cd")
                    for k, (col, cdst) in enumerate(
                            ((6, bcol), (7, acol), (8, tcol))):
                        sl = cd[:, CR * (k + 1):CR * (k + 2)]
                        nc.vector.tensor_copy(out=sl, in_=stv9[:, :, col])
                        nc.scalar.dma_start(out=cdst[:][s8], in_=sl)
                    # coords -> fp16 (x,y,z,0) quads
                    nc.vector.tensor_copy(out=cd[:, 0:CR], in_=stv9[:, :, 5])
                    cdh = cextp.tile([128, 4 * QPP], F16, tag="cdh")
                    nc.vector.memset(cdh[:], 0.0)
                    nc.vector.tensor_copy(
                        out=cdh[:].rearrange("p (a k) -> p a k", k=4)[:, :, 0:3],
                        in_=cd[:, 0:CR].rearrange("p (a k) -> p a k", k=3))
                    nc.scalar.dma_start(out=ccoord[:][s8], in_=cdh[:])

                # ------------- fp16-quad coord table ------------------------
                # partition 16g+j holds sample g's (x,y,z,0) fp16 quads
                tab = tabp.tile([128, 4 * 4096], F16, tag="tab")
                nc.sync.dma_start(
                    out=tab[:],
                    in_=ccoord[:][:, None, 0:4 * NATOMS].to_broadcast(
                        [8, 16, 4 * NATOMS]),
                )

                # ------------- per-class: indices -> gathers -> energy ------
                for (ccol, ne, ptab, pn, pp, acc_c) in (
                    (bcol, 2, pbt, 15, 2, 0),
                    (acol, 3, pat, 13, 2, 1),
                    (tcol, 4, ptt, 25, 4, 2),
                ):
                    cnt = (ne + 1) * DP
                    iraw = irawp.tile([128, cnt], F32, tag="iraw")
                    nc.scalar.dma_start(
                        out=iraw[:],
                        in_=ccol[:][:, 0:16 * cnt].rearrange("s (j f) -> s j f", j=16),
                    )
                    idxt = idxp.tile([128, (ne + 1) * DP], I16, tag="idxt")
                    irv = iraw[:].rearrange("p (e k) -> p e k", k=ne + 1)
                    for e in range(ne + 1):
                        nc.vector.tensor_copy(
                            out=idxt[:, DP * e:DP * (e + 1)], in_=irv[:, :, e])

                    npl = 3 * ne + pp
                    pl = densep.tile([128, npl * DP], F32, tag=f"pl{acc_c}")

                    def plv(n):
                        return pl[:, DP * n:DP * (n + 1)]

                    # coord gathers: one d=4 fp16-quad gather per endpoint
                    for e in range(ne):
                        gh = gathp.tile([128, 4 * LIST], F16, tag="gh")
                        nc.gpsimd.ap_gather(
                            out_ap=gh[:].rearrange("p (n d) -> p n d", d=4),
                            in_ap=tab[:].rearrange("p (n d) -> p n d", d=4),
                            idxs_ap=idxt[:, DP * e:DP * (e + 1)],
                            channels=128,
                            num_elems=4096,
                            d=4,
                            num_idxs=LIST,
                        )
                        dnh = dnp.tile([128, 4 * DP], F16, tag="dnh")
                        nc.sync.dma_start(
                            out=dnh[:],
                            in_=gh[:].rearrange("(a b) f -> a b f", b=16)[:, 0, :],
                        )
                        dv = dnh[:].rearrange("p (n d) -> p n d", d=4)
                        for r in range(3):
                            nc.vector.tensor_copy(out=plv(3 * e + r), in_=dv[:, :, r])
                    if pp == 4:
                        # torsion params: one d=4 fp16 (k, cosd, sind, n) gather
                        gh = gathp.tile([128, 4 * LIST], F16, tag="gh")
                        nc.gpsimd.ap_gather(
                            out_ap=gh[:].rearrange("p (n d) -> p n d", d=4),
                            in_ap=ptab[0].rearrange("p (n d) -> p n d", d=4),
                            idxs_ap=idxt[:, DP * ne:DP * (ne + 1)],
                            channels=128,
                            num_elems=pn,
                            d=4,
                            num_idxs=LIST,
                        )
                        dnh = dnp.tile([128, 4 * DP], F16, tag="dnh")
                        nc.sync.dma_start(
                            out=dnh[:],
                            in_=gh[:].rearrange("(a b) f -> a b f", b=16)[:, 0, :],
                        )
                        dv = dnh[:].rearrange("p (n d) -> p n d", d=4)
                        for r in range(4):
                            nc.vector.tensor_copy(
                                out=plv(3 * ne + r), in_=dv[:, :, r])
                    else:
                        # bond/angle params: d=2 fp16 (k, x0) gather
                        gh = gathp.tile([128, 4 * LIST], F16, tag="gh")
                        g2 = gh[:, 0:2 * LIST]
                        nc.gpsimd.ap_gather(
                            out_ap=g2.rearrange("p (n d) -> p n d", d=2),
                            in_ap=ptab[0].rearrange("p (n d) -> p n d", d=2),
                            idxs_ap=idxt[:, DP * ne:DP * (ne + 1)],
                            channels=128,
                            num_elems=pn,
                            d=2,
                            num_idxs=LIST,
                        )
                        dnh = dnp.tile([128, 4 * DP], F16, tag="dnh")
                        dn2 = dnh[:, 0:2 * DP]
                        nc.sync.dma_start(
                            out=dn2,
                            in_=g2.rearrange("(a b) f -> a b f", b=16)[:, 0, :],
                        )
                        dv = dn2.rearrange("p (n d) -> p n d", d=2)
                        for r in range(2):
                            nc.vector.tensor_copy(
                                out=plv(3 * ne + r), in_=dv[:, :, r])

                    acc = acc6[:, 3 * w + acc_c:3 * w + acc_c + 1]
                    if acc_c == 0:
                        bond_energy(nc, workp, plv, pl, mB, b_eps, acc)
                    elif acc_c == 1:
                        angle_energy(nc, workp, plv, pl, mA, b_eps, acc)
                    else:
                        torsion_energy(nc, workp, plv, pl, mT, b_eps, b_tiny, acc)

            # ------------- final reduction: [128, 6] -> [8, 6] -> out -------
            pacc = psump.tile([8, 6], F32, tag="pacc")
            nc.tensor.matmul(out=pacc[:], lhsT=blk, rhs=acc6[:], start=True, stop=True)
            nc.vector.tensor_copy(out=otmp[:], in_=pacc[:])
            nc.vector.tensor_mul(out=otmp[:], in0=otmp[:], in1=opt6[:])
            nc.sync.dma_start(out=out_d.ap()[0:8, :], in_=otmp[:, 0:3])
            nc.sync.dma_start(out=out_d.ap()[8:16, :], in_=otmp[:, 3:6])

    nc.compile()
    return nc


def bond_energy(nc, workp, plv, pl, mB, b_eps, acc):
    # planes: xi yi zi (0:3) xj yj zj (3:6) k (6) r0 (7)
    wb = workp.tile([128, 8 * DP], F32, tag="w8")
    d3 = wb[:, 0:3 * DP]
    sq = wb[:, 3 * DP:6 * DP]
    r2 = wb[:, 6 * DP:7 * DP]
    scr = wb[:, 7 * DP:8 * DP]
    nc.vector.tensor_sub(out=d3, in0=pl[:, 0:3 * DP], in1=pl[:, 3 * DP:6 * DP])
    nc.vector.tensor_mul(out=sq, in0=d3, in1=d3)
    nc.vector.tensor_reduce(
        out=r2, in_=sq.rearrange("p (c u) -> p u c", c=3), axis=AX.X, op=Op.add)
    r = wb[:, 0:DP]                       # d3 dead
    nc.scalar.activation(r, r2, AF.Sqrt, bias=b_eps)
    u = wb[:, DP:2 * DP]
    nc.vector.tensor_sub(out=u, in0=r, in1=plv(7))
    e = wb[:, 2 * DP:3 * DP]
    nc.scalar.activation(e, u, AF.Square)
    km = wb[:, 3 * DP:4 * DP]
    nc.vector.tensor_tensor(out=km, in0=plv(6), in1=mB, op=Op.mult)
    nc.vector.tensor_mul(out=scr, in0=e, in1=km)
    nc.vector.tensor_reduce(out=acc, in_=scr, axis=AX.X, op=Op.add)


def angle_energy(nc, workp, plv, pl, mA, b_eps, acc):
    # planes: i (0:3) j (3:6) k (6:9) k_t (9) th0 (10)
    wv = workp.tile([128, 6 * DP], F32, tag="w6")
    v1 = wv[:, 0:3 * DP]
    v2 = wv[:, 3 * DP:6 * DP]
    nc.vector.tensor_sub(out=v1, in0=pl[:, 0:3 * DP], in1=pl[:, 3 * DP:6 * DP])
    nc.vector.tensor_sub(out=v2, in0=pl[:, 6 * DP:9 * DP], in1=pl[:, 3 * DP:6 * DP])
    wa = workp.tile([128, 8 * DP], F32, tag="w8")
    prod = wa[:, 0:3 * DP]
    d11 = wa[:, 3 * DP:4 * DP]
    d22 = wa[:, 4 * DP:5 * DP]
    d12 = wa[:, 5 * DP:6 * DP]

    def dot3(dst, a, b):
        nc.vector.tensor_mul(out=prod, in0=a, in1=b)
        nc.vector.tensor_reduce(
            out=dst, in_=prod.rearrange("p (c u) -> p u c", c=3), axis=AX.X, op=Op.add)

    dot3(d11, v1, v1)
    dot3(d22, v2, v2)
    dot3(d12, v1, v2)
    s1 = wa[:, 0:DP]                      # prod dead
    s2a = wa[:, 1 * DP:2 * DP]
    nc.scalar.activation(s1, d11, AF.Sqrt, bias=b_eps)
    nc.scalar.activation(s2a, d22, AF.Sqrt, bias=b_eps)
    den = wa[:, 2 * DP:3 * DP]
    nc.vector.tensor_mul(out=den, in0=s1, in1=s2a)
    nc.vector.reciprocal(out=den, in_=den)
    cosv = wa[:, 3 * DP:4 * DP]           # d11 dead
    nc.vector.tensor_mul(out=cosv, in0=d12, in1=den)
    cosc = wa[:, 4 * DP:5 * DP]           # d22 dead
    nc.vector.tensor_scalar(
        out=cosc, in0=cosv, scalar1=-1.0 + 1e-6, scalar2=1.0 - 1e-6,
        op0=Op.max, op1=Op.min)
    # theta = arccos(cosc) via two bounded-arg arctan branches:
    #  |c| >  s: theta = arctan(s/c) + pi*(c<0)
    #  |c| <= s: theta = pi/2 - arctan(c/s), s = sqrt(1-c^2)
    cc = wa[:, 0:DP]                      # s1 dead
    nc.scalar.activation(cc, cosc, AF.Square)
    om = wa[:, 1 * DP:2 * DP]             # s2a dead
    nc.vector.tensor_scalar(
        out=om, in0=cc, scalar1=-1.0, scalar2=1.0, op0=Op.mult, op1=Op.add)
    sn = wa[:, 2 * DP:3 * DP]             # den dead
    nc.scalar.activation(sn, om, AF.Sqrt)
    sgn = wa[:, 1 * DP:2 * DP]            # om dead
    nc.vector.tensor_scalar(
        out=sgn, in0=cosc, scalar1=0.0, scalar2=None, op0=Op.is_ge)
    nc.vector.tensor_scalar(
        out=sgn, in0=sgn, scalar1=2e-18, scalar2=-1e-18, op0=Op.mult, op1=Op.add)
    csafe = wa[:, 5 * DP:6 * DP]          # d12 dead
    nc.vector.tensor_add(out=csafe, in0=cosc, in1=sgn)
    nc.vector.reciprocal(out=csafe, in_=csafe)
    ra = wa[:, 1 * DP:2 * DP]             # sgn dead
    nc.vector.tensor_mul(out=ra, in0=sn, in1=csafe)
    nc.vector.tensor_scalar(
        out=ra, in0=ra, scalar1=-1.0, scalar2=1.0, op0=Op.max, op1=Op.min)
    ata = wa[:, 5 * DP:6 * DP]            # csafe dead
    nc.scalar.activation(ata, ra, AF.Arctan)
    corr = wa[:, 1 * DP:2 * DP]           # ra dead
    nc.vector.tensor_scalar(
        out=corr, in0=cosc, scalar1=0.0, scalar2=PI, op0=Op.is_lt, op1=Op.mult)
    tha = wa[:, 6 * DP:7 * DP]
    nc.vector.tensor_add(out=tha, in0=ata, in1=corr)
    nc.vector.reciprocal(out=sn, in_=sn)
    rb = wa[:, 1 * DP:2 * DP]             # corr dead
    nc.vector.tensor_mul(out=rb, in0=cosc, in1=sn)
    nc.vector.tensor_scalar(
        out=rb, in0=rb, scalar1=-1.0, scalar2=1.0, op0=Op.max, op1=Op.min)
    thb = wa[:, 5 * DP:6 * DP]            # ata dead
    nc.scalar.activation(thb, rb, AF.Arctan)
    nc.vector.tensor_scalar(
        out=thb, in0=thb, scalar1=-1.0, scalar2=PI / 2.0, op0=Op.mult, op1=Op.add)
    wi = workp.tile([128, DP], I32, tag="wi")
    nc.vector.tensor_scalar(
        out=wi[:], in0=cc, scalar1=0.5, scalar2=None, op0=Op.is_gt)
    th = wa[:, 7 * DP:8 * DP]
    nc.vector.select(out=th, mask=wi[:], on_true=tha, on_false=thb)
    ua = wa[:, 0:DP]                      # cc dead
    nc.vector.tensor_sub(out=ua, in0=th, in1=plv(10))
    ea = wa[:, 1 * DP:2 * DP]             # rb dead
    nc.scalar.activation(ea, ua, AF.Square)
    kma = wa[:, 2 * DP:3 * DP]            # sn dead
    nc.vector.tensor_tensor(out=kma, in0=plv(9), in1=mA, op=Op.mult)
    scr = wa[:, 3 * DP:4 * DP]            # cosv dead
    nc.vector.tensor_mul(out=scr, in0=ea, in1=kma)
    nc.vector.tensor_reduce(out=acc, in_=scr, axis=AX.X, op=Op.add)


def torsion_energy(nc, workp, plv, pl, mT, b_eps, b_tiny, acc):
    # planes: i (0:3) j (3:6) k (6:9) l (9:12) k_t (12) cosd (13) sind (14) n (15)
    wv = workp.tile([128, 6 * DP], F32, tag="w6")
    wb9 = workp.tile([128, 9 * DP], F32, tag="w9")
    b1 = wb9[:, 0:3 * DP]
    b2 = wb9[:, 3 * DP:6 * DP]
    b3 = wb9[:, 6 * DP:9 * DP]
    nc.vector.tensor_sub(out=b1, in0=pl[:, 3 * DP:6 * DP], in1=pl[:, 0:3 * DP])
    nc.vector.tensor_sub(out=b2, in0=pl[:, 6 * DP:9 * DP], in1=pl[:, 3 * DP:6 * DP])
    nc.vector.tensor_sub(out=b3, in0=pl[:, 9 * DP:12 * DP], in1=pl[:, 6 * DP:9 * DP])

    def comp(t, m):
        return t[:, DP * m:DP * (m + 1)]

    # n1 = b1 x b2 -> wv[0:3]; n2 = b2 x b3 -> wv[3:6]
    wt = workp.tile([128, 8 * DP], F32, tag="w8")
    t0 = wt[:, 0:DP]
    t1 = wt[:, DP:2 * DP]
    for m in range(3):
        m1_, m2_ = (m + 1) % 3, (m + 2) % 3
        nc.vector.tensor_mul(out=t0, in0=comp(b1, m1_), in1=comp(b2, m2_))
        nc.vector.tensor_mul(out=t1, in0=comp(b1, m2_), in1=comp(b2, m1_))
        nc.vector.tensor_sub(out=comp(wv, m), in0=t0, in1=t1)
        nc.vector.tensor_mul(out=t0, in0=comp(b2, m1_), in1=comp(b3, m2_))
        nc.vector.tensor_mul(out=t1, in0=comp(b2, m2_), in1=comp(b3, m1_))
        nc.vector.tensor_sub(out=comp(wv, 3 + m), in0=t0, in1=t1)
    n1 = wv[:, 0:3 * DP]
    n2 = wv[:, 3 * DP:6 * DP]
    # q2 = |b2|^2 (b1 slot becomes scratch after m1 computed; use b3 later)
    q2 = wt[:, 2 * DP:3 * DP]
    sq = workp.tile([128, 3 * DP], F32, tag="w3")
    nc.vector.tensor_mul(out=sq[:], in0=b2, in1=b2)
    nc.vector.tensor_reduce(
        out=q2, in_=sq[:].rearrange("p (c u) -> p u c", c=3), axis=AX.X, op=Op.add)
    # m1' = n1 x b2 -> b3 slot (b3 dead after n2)
    for m in range(3):
        m1_, m2_ = (m + 1) % 3, (m + 2) % 3
        nc.vector.tensor_mul(out=t0, in0=comp(n1, m1_), in1=comp(b2, m2_))
        nc.vector.tensor_mul(out=t1, in0=comp(n1, m2_), in1=comp(b2, m1_))
        nc.vector.tensor_sub(out=comp(b3, m), in0=t0, in1=t1)
    mp = b3
    # X = n1.n2 ; Yr = m1'.n2
    X = wt[:, 3 * DP:4 * DP]
    Yr = wt[:, 4 * DP:5 * DP]
    nc.vector.tensor_mul(out=sq[:], in0=n1, in1=n2)
    nc.vector.tensor_reduce(
        out=X, in_=sq[:].rearrange("p (c u) -> p u c", c=3), axis=AX.X, op=Op.add)
    nc.vector.tensor_mul(out=sq[:], in0=mp, in1=n2)
    nc.vector.tensor_reduce(
        out=Yr, in_=sq[:].rearrange("p (c u) -> p u c", c=3), axis=AX.X, op=Op.add)
    rn = wt[:, 5 * DP:6 * DP]
    nc.scalar.activation(rn, q2, AF.Sqrt, bias=b_eps)
    nc.vector.reciprocal(out=rn, in_=rn)
    y = wt[:, 6 * DP:7 * DP]
    nc.vector.tensor_mul(out=y, in0=Yr, in1=rn)
    hx = wt[:, 2 * DP:3 * DP]             # q2 dead
    hy = wt[:, 4 * DP:5 * DP]             # Yr dead
    nc.scalar.activation(hx, X, AF.Square)
    nc.scalar.activation(hy, y, AF.Square)
    h = wt[:, 5 * DP:6 * DP]              # rn dead
    nc.vector.tensor_add(out=h, in0=hx, in1=hy)
    rh = wt[:, 2 * DP:3 * DP]             # hx dead
    nc.scalar.activation(rh, h, AF.Sqrt, bias=b_tiny)
    nc.vector.reciprocal(out=rh, in_=rh)
    c = wt[:, 4 * DP:5 * DP]              # hy dead
    s = wt[:, 5 * DP:6 * DP]              # h dead
    nc.vector.tensor_mul(out=c, in0=X, in1=rh)
    nc.vector.tensor_mul(out=s, in0=y, in1=rh)
    # Chebyshev: cos/sin of 2phi, 3phi (scratch: sq planes + X/rh/t0/t1 slots)
    cc = comp(sq, 0)
    c2 = comp(sq, 1)
    s2 = comp(sq, 2)
    sc = wt[:, 3 * DP:4 * DP]             # X dead
    c3 = wt[:, 2 * DP:3 * DP]             # rh dead
    s3 = wt[:, 6 * DP:7 * DP]             # y dead
    nc.scalar.activation(cc, c, AF.Square)
    nc.vector.tensor_scalar(
        out=c2, in0=cc, scalar1=2.0, scalar2=-1.0, op0=Op.mult, op1=Op.add)
    nc.vector.tensor_mul(out=sc, in0=s, in1=c)
    nc.vector.tensor_scalar(
        out=s2, in0=sc, scalar1=2.0, scalar2=None, op0=Op.mult)
    nc.vector.tensor_scalar(
        out=t0, in0=cc, scalar1=4.0, scalar2=-3.0, op0=Op.mult, op1=Op.add)
    nc.vector.tensor_mul(out=c3, in0=t0, in1=c)
    nc.vector.tensor_scalar(
        out=t0, in0=cc, scalar1=4.0, scalar2=-1.0, op0=Op.mult, op1=Op.add)
    nc.vector.tensor_mul(out=s3, in0=t0, in1=s)
    wi = workp.tile([128, 2 * DP], I32, tag="wi2")
    m2m = wi[:, 0:DP]
    m3m = wi[:, DP:2 * DP]
    nc.vector.tensor_scalar(
        out=m2m, in0=plv(15), scalar1=2.0, scalar2=None, op0=Op.is_equal)
    nc.vector.tensor_scalar(
        out=m3m, in0=plv(15), scalar1=3.0, scalar2=None, op0=Op.is_equal)
    cn = wt[:, 0:DP]                      # t0 dead
    sn = wt[:, 1 * DP:2 * DP]             # t1 dead
    nc.vector.select(out=cn, mask=m2m, on_true=c2, on_false=c)
    nc.vector.select(out=cn, mask=m3m, on_true=c3, on_false=cn)
    nc.vector.select(out=sn, mask=m2m, on_true=s2, on_false=s)
    nc.vector.select(out=sn, mask=m3m, on_true=s3, on_false=sn)
    tt1 = wt[:, 3 * DP:4 * DP]            # sc dead
    tt2 = wt[:, 4 * DP:5 * DP]            # c dead
    nc.vector.tensor_mul(out=tt1, in0=cn, in1=plv(13))
    nc.vector.tensor_mul(out=tt2, in0=sn, in1=plv(14))
    esum = wt[:, 5 * DP:6 * DP]           # s dead
    nc.vector.tensor_add(out=esum, in0=tt1, in1=tt2)
    nc.vector.tensor_scalar(
        out=esum, in0=esum, scalar1=1.0, scalar2=None, op0=Op.add)
    kmt = wt[:, 6 * DP:7 * DP]            # c3... s3 dead
    nc.vector.tensor_tensor(out=kmt, in0=plv(12), in1=mT, op=Op.mult)
    scr = wt[:, 7 * DP:8 * DP]
    nc.vector.tensor_mul(out=scr, in0=esum, in1=kmt)
    nc.vector.tensor_reduce(out=acc, in_=scr, axis=AX.X, op=Op.add)


@functools.lru_cache(maxsize=1)
def _get_nc():
    return build_nc()


def make_in_maps(inputs):
    """Shard full inputs into 8 per-core input maps."""
    feats = np.ascontiguousarray(inputs["features"], dtype=np.float32)
    Bf = feats.shape[0]
    flat = feats.reshape(Bf, -1)
    flat = np.concatenate(
        [flat, np.zeros((Bf, FLATPAD - flat.shape[1]), np.float32)], axis=1
    )
    bond_type = np.ascontiguousarray(inputs["bond_type"], np.float32)
    angle_type = np.ascontiguousarray(inputs["angle_type"], np.float32)
    tor_type = np.ascontiguousarray(inputs["tor_type"], np.float32)
    mult_f = np.ascontiguousarray(inputs["multiplicity"], np.float32).reshape(1, 25)
    opt = np.ascontiguousarray(inputs["opt_pars"], np.float32).reshape(1, 47)
    n_nc = Bf // NS
    in_maps = []
    for k in range(n_nc):
        in_maps.append({
            "features": flat[NS * k:NS * (k + 1)],
            "bond_type": bond_type,
            "angle_type": angle_type,
            "tor_type": tor_type,
            "mult_f": mult_f,
            "opt_pars": opt,
        })
    return in_maps


def kernel(**inputs) -> np.ndarray:
    from concourse.bass_utils import run_bass_kernel_spmd

    nc = _get_nc()
    in_maps = make_in_maps(inputs)
    res = run_bass_kernel_spmd(nc, in_maps, core_ids=list(range(len(in_maps))))
    outs = [res.results[k]["out"] for k in range(len(in_maps))]
    return np.concatenate(outs, axis=0).astype(np.float32)


def simulate_one_core(inputs, nc=None, trace=False):
    """CoreSim a single NC on the first 16 samples (for correctness dev)."""
    import concourse.bass_interp as bass_interp

    if nc is None:
        nc = _get_nc()
    in_map = make_in_maps(inputs)[0]
    sim = bass_interp.MultiCoreSim(nc, 1, trace=trace)
    for name, val in in_map.items():
        sim.cores[0].tensor(name)[:] = val
    sim.simulate(check_with_hw=False)
    return np.array(sim.cores[0].mem_tensor("out"))


if __name__ == "__main__":
    nc = build_nc()
    print("build ok")
